# revision 2
# baseline (speedup 1.0000x reference)
"""FERNN cell kernel for 8x Trainium2 NeuronCores (Bass/Tile).

Computation (per sample b):
    u_conv = circ_conv(u_t, w_u)                      # [CH, 64, 64]
    u_full = zero-pad to [CH, 128, 128] (top-left)
    h_shift[c,i,j] = h_prev[c,(i+dy)%128,(j+dx)%128]  # (dx,dy) = action[b]
    out = relu(u_full + circ_conv(h_shift, w_h))

Strategy: data-parallel over batch (4 samples / core).  The per-sample roll
and the circular conv padding are folded into a host-side circular halo pad
of h_prev to [137,137]; on device each sample's conv input window is read at
a *dynamic* offset (dy, dx) loaded from the action tensor, so the conv
directly produces the rolled h_conv.  u_conv accumulates into the same PSUM
tile (it lands at static positions post-roll), and a fused ReLU drain
streams blocks straight back to DRAM.

Conv = 9 accumulating matmuls per output block (N=512, K=128) in float32r
(full PE rate, fp32 precision).
"""

import numpy as np

B, CIN, CH = 32, 3, 128
WIN, WORLD, K = 64, 128, 3
NCORES = 8
BLOC = B // NCORES          # samples per core
HP = WORLD + 9              # host-padded h: rows/cols -1 .. 135  -> 137
UP = WIN + 2                # host-padded u: 66

_prog_cache = {}
TRACE = False
MODE = "f32r"
HQ_BUFS = 3
OROW_BUFS = 4
TAP_OUTER = True
IM_BUFS = 2
PSUM_BUFS = 8
LAST_RESULTS = None


def _round_tf32(a):
    """Round-to-nearest-even fp32 -> tf32 (10-bit mantissa), as float32."""
    a = np.ascontiguousarray(a, np.float32)
    u = a.view(np.uint32)
    lsb = (u >> np.uint32(13)) & np.uint32(1)
    r = (u + np.uint32(0x0FFF) + lsb) & np.uint32(0xFFFFE000)
    return r.view(np.float32)


def _build_program(mode="f32r", repeat=1):
    import concourse.bass as bass
    import concourse.tile as tile
    from concourse import bacc, mybir

    f32 = mybir.dt.float32
    f32r = mybir.dt.float32r
    bf16 = mybir.dt.bfloat16
    i32 = mybir.dt.int32
    ds = bass.ds

    in_dt = bf16 if mode == "bf16" else (f32r if mode == "f32r" else f32)

    nc = bacc.Bacc(
        "TRN2",
        target_bir_lowering=False,
        debug=False,
        enable_asserts=False,
        num_devices=NCORES,
    )

    h = nc.dram_tensor("h", [BLOC * CH, HP, HP], in_dt, kind="ExternalInput")
    u = nc.dram_tensor("u", [BLOC * CIN, UP, UP], in_dt, kind="ExternalInput")
    act = nc.dram_tensor("act", [1, 2 * BLOC], i32, kind="ExternalInput")
    wh = nc.dram_tensor("wh", [CH, 9 * CH], in_dt, kind="ExternalInput")
    wu = nc.dram_tensor("wu", [9 * CIN, CH], in_dt, kind="ExternalInput")
    out = nc.dram_tensor("out", [BLOC * CH, WORLD, WORLD], f32, kind="ExternalOutput")

    h_ap, u_ap, act_ap, wh_ap, wu_ap, out_ap = (
        t.ap() for t in (h, u, act, wh, wu, out)
    )

    def mm_cast(ap):
        return ap

    reps = repeat
    CHUNK = 33 * HP + WORLD + 2          # gathered elems per partition per quarter
    QSTEP = 32 * HP                      # element step between quarter row-starts
    h_flat = h.ap().rearrange("p a b -> p (a b)")

    with tile.TileContext(nc) as tc:
        with (
            tc.tile_pool(name="consts", bufs=1) as consts,
            tc.tile_pool(name="offs", bufs=2 * BLOC) as offs_pool,
            tc.tile_pool(name="hq", bufs=HQ_BUFS) as hq_pool,
            tc.tile_pool(name="im2col", bufs=IM_BUFS) as im_pool,
            tc.tile_pool(name="orow", bufs=OROW_BUFS) as out_pool,
            tc.tile_pool(name="psum", bufs=PSUM_BUFS, space="PSUM") as psum_pool,
        ):
            wh_t = consts.tile([CH, 9 * CH], in_dt)
            nc.sync.dma_start(wh_t[:], wh_ap[:])
            wu_t = consts.tile([9 * CIN, CH], in_dt)
            nc.sync.dma_start(wu_t[:], wu_ap[:])
            # action broadcast to all partitions via 0-stride DMA read
            act_bc = consts.tile([128, 2 * BLOC], i32)
            nc.sync.dma_start(act_bc[:], bass.AP(act, 0, [[0, 128], [1, 2 * BLOC]]))

            for _rep in range(reps):
              for b in range(BLOC):
                # gather offsets: offs[ci, q] = (b*CH+ci)*HP*HP + (dy+32q)*HP + dx
                qbase = offs_pool.tile([128, 4], i32, tag="qbase")
                nc.gpsimd.iota(
                    qbase[:], pattern=[[QSTEP, 4]], base=b * CH * HP * HP,
                    channel_multiplier=HP * HP,
                )
                dyx = offs_pool.tile([128, 1], i32, tag="dyx")
                nc.vector.tensor_scalar(
                    out=dyx[:], in0=act_bc[:, 2 * b + 1 : 2 * b + 2],
                    scalar1=HP, scalar2=None, op0=mybir.AluOpType.mult,
                )
                nc.vector.tensor_tensor(
                    out=dyx[:], in0=dyx[:], in1=act_bc[:, 2 * b : 2 * b + 1],
                    op=mybir.AluOpType.add,
                )
                offs = offs_pool.tile([128, 4], i32, tag="offs")
                nc.vector.tensor_tensor(
                    out=offs[:], in0=qbase[:], in1=dyx[:].to_broadcast([128, 4]),
                    op=mybir.AluOpType.add,
                )

                # im2col for the u-window conv: [27, 64*64]
                imc = im_pool.tile([9 * CIN, WIN * WIN], in_dt)
                for t in range(9):
                    di, dj = t // 3, t % 3
                    nc.sync.dma_start(
                        imc[3 * t : 3 * t + 3, :],
                        u_ap[b * CIN : (b + 1) * CIN, di : di + WIN, dj : dj + WIN],
                    )

                for q in range(4):
                    # gather the rolled+halo'd conv input window for this quarter:
                    # hq[ci, a, c] = hpad[b, ci, dy + 32q + a, dx + c]
                    hq = hq_pool.tile([CH, 34, HP], in_dt)
                    hq_flat = hq[:].rearrange("p a c -> p (a c)")
                    nc.gpsimd.indirect_dma_start(
                        out=hq_flat[:, 0:CHUNK],
                        out_offset=None,
                        in_=h_flat,
                        in_offset=bass.IndirectOffsetOnAxis(ap=offs[:, q : q + 1], axis=1),
                    )

                    if not TAP_OUTER:
                      for k in range(4):
                        i0 = 32 * q + 8 * k
                        orow = out_pool.tile([CH, 8, WORLD], f32)
                        for half in range(2):
                            c0 = 64 * half
                            with_u = half == 0 and i0 < WIN
                            ps = psum_pool.tile([CH, 8, 64], f32)
                            for t in range(9):
                                di, dj = t // 3, t % 3
                                rhs = hq[
                                    :,
                                    8 * k + di : 8 * k + di + 8,
                                    c0 + dj : c0 + dj + 64,
                                ]
                                nc.tensor.matmul(
                                    ps[:],
                                    wh_t[:, t * CH : (t + 1) * CH],
                                    rhs,
                                    start=(t == 0),
                                    stop=(t == 8 and not with_u),
                                )
                            if with_u:
                                nc.tensor.matmul(
                                    ps[:],
                                    wu_t[:],
                                    imc[:, i0 * 64 : (i0 + 8) * 64],
                                    start=False,
                                    stop=True,
                                )
                            # fused relu drain PSUM -> SBUF out rows
                            dst = orow[:, :, c0 : c0 + 64]
                            if half == 0:
                                nc.scalar.activation(
                                    dst, ps[:], mybir.ActivationFunctionType.Relu
                                )
                            else:
                                nc.vector.tensor_scalar_max(dst, ps[:], 0.0)
                        nc.sync.dma_start(
                            out_ap[b * CH : (b + 1) * CH, i0 : i0 + 8, :], orow[:]
                        )
                    else:
                      # tap-outer: reuse each tap weight across all 8 psum
                      # blocks of the quarter before switching taps
                      pss = [psum_pool.tile([CH, 8, 64], f32, name=f"pss{_j}", tag="ps_to") for _j in range(8)]
                      orows = [out_pool.tile([CH, 8, WORLD], f32, name=f"orows{_j}", tag="orow_to") for _j in range(4)]
                      for t in range(9):
                          di, dj = t // 3, t % 3
                          for k in range(4):
                              for half in range(2):
                                  c0 = 64 * half
                                  i0 = 32 * q + 8 * k
                                  with_u = half == 0 and i0 < WIN
                                  rhs = hq[
                                      :,
                                      8 * k + di : 8 * k + di + 8,
                                      c0 + dj : c0 + dj + 64,
                                  ]
                                  nc.tensor.matmul(
                                      pss[2 * k + half][:],
                                      wh_t[:, t * CH : (t + 1) * CH],
                                      rhs,
                                      start=(t == 0),
                                      stop=(t == 8 and not with_u),
                                  )
                      for k in range(4):
                          i0 = 32 * q + 8 * k
                          if i0 < WIN:
                              nc.tensor.matmul(
                                  pss[2 * k][:],
                                  wu_t[:],
                                  imc[:, i0 * 64 : (i0 + 8) * 64],
                                  start=False,
                                  stop=True,
                              )
                      for k in range(4):
                          i0 = 32 * q + 8 * k
                          nc.scalar.activation(
                              orows[k][:, :, 0:64], pss[2 * k][:],
                              mybir.ActivationFunctionType.Relu,
                          )
                          nc.vector.tensor_scalar_max(
                              orows[k][:, :, 64:128], pss[2 * k + 1][:], 0.0
                          )
                          nc.sync.dma_start(
                              out_ap[b * CH : (b + 1) * CH, i0 : i0 + 8, :],
                              orows[k][:],
                          )

    nc.compile()
    return nc


def _get_program(mode, repeat=1):
    key = (mode, repeat)
    if key not in _prog_cache:
        _prog_cache[key] = _build_program(mode, repeat)
    return _prog_cache[key]


def _host_prep(u_t, h_prev, action, w_u, w_h, mode):
    """Host-side layout prep shared by kernel() and the timing harness."""
    np_in = np.float32
    if mode == "bf16":
        import ml_dtypes

        np_in = ml_dtypes.bfloat16

    # host-side circular halo pads (pure layout; all compute stays on device)
    hpad = np.pad(h_prev, ((0, 0), (0, 0), (1, 8), (1, 8)), mode="wrap")
    upad = np.pad(u_t, ((0, 0), (0, 0), (1, 1), (1, 1)), mode="wrap")
    # weight layouts for matmul lhsT (stationary [K, M])
    wh_l = np.ascontiguousarray(
        w_h.transpose(1, 2, 3, 0).reshape(CH, 9 * CH)
    )  # [ci, (di*3+dj)*128+co]
    wu_l = np.ascontiguousarray(
        w_u.transpose(2, 3, 1, 0).reshape(9 * CIN, CH)
    )  # [(di*3+dj)*3+ci, co]

    if mode == "f32r":
        hpad = _round_tf32(hpad)
        upad = _round_tf32(upad)
        wh_l = _round_tf32(wh_l)
        wu_l = _round_tf32(wu_l)
    else:
        hpad = hpad.astype(np_in)
        upad = upad.astype(np_in)
        wh_l = wh_l.astype(np_in)
        wu_l = wu_l.astype(np_in)
    return {"hpad": hpad, "upad": upad, "act": action, "wh": wh_l, "wu": wu_l}


def _core_in_map(prep, c):
    s = slice(c * BLOC, (c + 1) * BLOC)
    return {
        "h": np.ascontiguousarray(prep["hpad"][s]).reshape(BLOC * CH, HP, HP),
        "u": np.ascontiguousarray(prep["upad"][s]).reshape(BLOC * CIN, UP, UP),
        "act": np.ascontiguousarray(prep["act"][s]).reshape(1, 2 * BLOC),
        "wh": prep["wh"],
        "wu": prep["wu"],
    }


def kernel(u_t, h_prev, action, w_u, w_h):
    global LAST_RESULTS
    from concourse.bass_utils import run_bass_kernel_spmd

    mode = MODE
    nc = _get_program(mode)

    u_t = np.asarray(u_t, np.float32)
    h_prev = np.asarray(h_prev, np.float32)
    action = np.asarray(action, np.int32)
    w_u = np.asarray(w_u, np.float32)
    w_h = np.asarray(w_h, np.float32)

    prep = _host_prep(u_t, h_prev, action, w_u, w_h, mode)
    in_maps = [_core_in_map(prep, c) for c in range(NCORES)]

    res = run_bass_kernel_spmd(nc, in_maps, list(range(NCORES)), trace=TRACE)
    LAST_RESULTS = res
    out = np.concatenate(
        [r["out"].reshape(BLOC, CH, WORLD, WORLD) for r in res.results], axis=0
    )
    return out



# revision 8
# speedup vs baseline: 1.2270x; 1.2270x over previous
"""FERNN cell kernel for 8x Trainium2 NeuronCores (Bass/Tile).

Computation (per sample b):
    u_conv = circ_conv(u_t, w_u)                      # [CH, 64, 64]
    u_full = zero-pad to [CH, 128, 128] (top-left)
    h_shift[c,i,j] = h_prev[c,(i+dy)%128,(j+dx)%128]  # (dx,dy) = action[b]
    out = relu(u_full + circ_conv(h_shift, w_h))

Strategy: data-parallel over batch (4 samples / core).  The per-sample roll
and the circular conv padding are folded into a host-side circular halo pad
of h_prev to [137,137]; on device each sample's conv input window is read at
a *dynamic* offset (dy, dx) loaded from the action tensor, so the conv
directly produces the rolled h_conv.  u_conv accumulates into the same PSUM
tile (it lands at static positions post-roll), and a fused ReLU drain
streams blocks straight back to DRAM.

Conv = 9 accumulating matmuls per output block (N=512, K=128) in float32r
(full PE rate, fp32 precision).
"""

import numpy as np

B, CIN, CH = 32, 3, 128
WIN, WORLD, K = 64, 128, 3
NCORES = 8
BLOC = B // NCORES          # samples per core
HP = WORLD + 9              # host-padded h: rows/cols -1 .. 135  -> 137
UP = WIN + 2                # host-padded u: 66

_prog_cache = {}
TRACE = False
MODE = "bf16"
OUT_BF16 = True
HQ_BUFS = 3
OROW_BUFS = 4
TAP_OUTER = True
IM_BUFS = 2
PSUM_BUFS = 8
LAST_RESULTS = None


def _round_tf32(a):
    """Round-to-nearest-even fp32 -> tf32 (10-bit mantissa), as float32."""
    a = np.ascontiguousarray(a, np.float32)
    u = a.view(np.uint32)
    lsb = (u >> np.uint32(13)) & np.uint32(1)
    r = (u + np.uint32(0x0FFF) + lsb) & np.uint32(0xFFFFE000)
    return r.view(np.float32)


def _build_program(mode="f32r", repeat=1):
    import concourse.bass as bass
    import concourse.tile as tile
    from concourse import bacc, mybir

    f32 = mybir.dt.float32
    f32r = mybir.dt.float32r
    bf16 = mybir.dt.bfloat16
    i32 = mybir.dt.int32
    ds = bass.ds

    in_dt = bf16 if mode == "bf16" else (f32r if mode == "f32r" else f32)

    nc = bacc.Bacc(
        "TRN2",
        target_bir_lowering=False,
        debug=False,
        enable_asserts=False,
        num_devices=NCORES,
    )

    out_dt = bf16 if OUT_BF16 else f32

    h = nc.dram_tensor("h", [BLOC * CH, HP, HP], in_dt, kind="ExternalInput")
    u = nc.dram_tensor("u", [BLOC * 9 * CIN, WIN * WIN], in_dt, kind="ExternalInput")
    act = nc.dram_tensor("act", [1, 2 * BLOC], i32, kind="ExternalInput")
    wh = nc.dram_tensor("wh", [CH, 9 * CH], in_dt, kind="ExternalInput")
    wu = nc.dram_tensor("wu", [9 * CIN, CH], in_dt, kind="ExternalInput")
    out = nc.dram_tensor("out", [BLOC * CH, WORLD, WORLD], out_dt, kind="ExternalOutput")

    h_ap, u_ap, act_ap, wh_ap, wu_ap, out_ap = (
        t.ap() for t in (h, u, act, wh, wu, out)
    )

    def mm_cast(ap):
        return ap

    reps = repeat
    CHUNK = 33 * HP + WORLD + 2          # gathered elems per partition per quarter
    QSTEP = 32 * HP                      # element step between quarter row-starts
    h_flat = h.ap().rearrange("p a b -> p (a b)")

    with tile.TileContext(nc) as tc:
        with (
            tc.tile_pool(name="consts", bufs=1) as consts,
            tc.tile_pool(name="offs", bufs=2 * BLOC) as offs_pool,
            tc.tile_pool(name="hq", bufs=HQ_BUFS) as hq_pool,
            tc.tile_pool(name="im2col", bufs=IM_BUFS) as im_pool,
            tc.tile_pool(name="orow", bufs=OROW_BUFS) as out_pool,
            tc.tile_pool(name="psum", bufs=PSUM_BUFS, space="PSUM") as psum_pool,
        ):
            wh_t = consts.tile([CH, 9 * CH], in_dt)
            nc.sync.dma_start(wh_t[:], wh_ap[:])
            wu_t = consts.tile([9 * CIN, CH], in_dt)
            nc.sync.dma_start(wu_t[:], wu_ap[:])
            # action broadcast to all partitions via 0-stride DMA read
            act_bc = consts.tile([128, 2 * BLOC], i32)
            nc.sync.dma_start(act_bc[:], bass.AP(act, 0, [[0, 128], [1, 2 * BLOC]]))

            for _rep in range(reps):
              for b in range(BLOC):
                # gather offsets: offs[ci, q] = (b*CH+ci)*HP*HP + (dy+32q)*HP + dx
                qbase = offs_pool.tile([128, 4], i32, tag="qbase")
                nc.gpsimd.iota(
                    qbase[:], pattern=[[QSTEP, 4]], base=b * CH * HP * HP,
                    channel_multiplier=HP * HP,
                )
                dyx = offs_pool.tile([128, 1], i32, tag="dyx")
                nc.vector.tensor_scalar(
                    out=dyx[:], in0=act_bc[:, 2 * b + 1 : 2 * b + 2],
                    scalar1=HP, scalar2=None, op0=mybir.AluOpType.mult,
                )
                nc.vector.tensor_tensor(
                    out=dyx[:], in0=dyx[:], in1=act_bc[:, 2 * b : 2 * b + 1],
                    op=mybir.AluOpType.add,
                )
                offs = offs_pool.tile([128, 4], i32, tag="offs")
                nc.vector.tensor_tensor(
                    out=offs[:], in0=qbase[:], in1=dyx[:].to_broadcast([128, 4]),
                    op=mybir.AluOpType.add,
                )

                # im2col for the u-window conv (host-prepared layout): [27, 64*64]
                imc = im_pool.tile([9 * CIN, WIN * WIN], in_dt)
                nc.sync.dma_start(
                    imc[:], u_ap[b * 9 * CIN : (b + 1) * 9 * CIN, :]
                )

                for q in range(4):
                    # gather the rolled+halo'd conv input window for this quarter:
                    # hq[ci, a, c] = hpad[b, ci, dy + 32q + a, dx + c]
                    hq = hq_pool.tile([CH, 34, HP], in_dt)
                    hq_flat = hq[:].rearrange("p a c -> p (a c)")
                    nc.gpsimd.indirect_dma_start(
                        out=hq_flat[:, 0:CHUNK],
                        out_offset=None,
                        in_=h_flat,
                        in_offset=bass.IndirectOffsetOnAxis(ap=offs[:, q : q + 1], axis=1),
                    )

                    if not TAP_OUTER:
                      for k in range(4):
                        i0 = 32 * q + 8 * k
                        orow = out_pool.tile([CH, 8, WORLD], out_dt)
                        for half in range(2):
                            c0 = 64 * half
                            with_u = half == 0 and i0 < WIN
                            ps = psum_pool.tile([CH, 8, 64], f32)
                            for t in range(9):
                                di, dj = t // 3, t % 3
                                rhs = hq[
                                    :,
                                    8 * k + di : 8 * k + di + 8,
                                    c0 + dj : c0 + dj + 64,
                                ]
                                nc.tensor.matmul(
                                    ps[:],
                                    wh_t[:, t * CH : (t + 1) * CH],
                                    rhs,
                                    start=(t == 0),
                                    stop=(t == 8 and not with_u),
                                )
                            if with_u:
                                nc.tensor.matmul(
                                    ps[:],
                                    wu_t[:],
                                    imc[:, i0 * 64 : (i0 + 8) * 64],
                                    start=False,
                                    stop=True,
                                )
                            # fused relu drain PSUM -> SBUF out rows
                            dst = orow[:, :, c0 : c0 + 64]
                            if half == 0:
                                nc.scalar.activation(
                                    dst, ps[:], mybir.ActivationFunctionType.Relu
                                )
                            else:
                                nc.vector.tensor_scalar_max(dst, ps[:], 0.0)
                        nc.sync.dma_start(
                            out_ap[b * CH : (b + 1) * CH, i0 : i0 + 8, :], orow[:]
                        )
                    else:
                      # tap-outer: reuse each tap weight across all 8 psum
                      # blocks of the quarter before switching taps
                      pss = [psum_pool.tile([CH, 8, 64], f32, name=f"pss{_j}", tag="ps_to") for _j in range(8)]
                      orows = [out_pool.tile([CH, 8, WORLD], out_dt, name=f"orows{_j}", tag="orow_to") for _j in range(4)]
                      for t in range(9):
                          di, dj = t // 3, t % 3
                          for k in range(4):
                              for half in range(2):
                                  c0 = 64 * half
                                  i0 = 32 * q + 8 * k
                                  with_u = half == 0 and i0 < WIN
                                  rhs = hq[
                                      :,
                                      8 * k + di : 8 * k + di + 8,
                                      c0 + dj : c0 + dj + 64,
                                  ]
                                  nc.tensor.matmul(
                                      pss[2 * k + half][:],
                                      wh_t[:, t * CH : (t + 1) * CH],
                                      rhs,
                                      start=(t == 0),
                                      stop=(t == 8 and not with_u),
                                  )
                      for k in range(4):
                          i0 = 32 * q + 8 * k
                          if i0 < WIN:
                              nc.tensor.matmul(
                                  pss[2 * k][:],
                                  wu_t[:],
                                  imc[:, i0 * 64 : (i0 + 8) * 64],
                                  start=False,
                                  stop=True,
                              )
                      for k in range(4):
                          i0 = 32 * q + 8 * k
                          nc.scalar.activation(
                              orows[k][:, :, 0:64], pss[2 * k][:],
                              mybir.ActivationFunctionType.Relu,
                          )
                          nc.vector.tensor_scalar_max(
                              orows[k][:, :, 64:128], pss[2 * k + 1][:], 0.0
                          )
                          nc.sync.dma_start(
                              out_ap[b * CH : (b + 1) * CH, i0 : i0 + 8, :],
                              orows[k][:],
                          )

    nc.compile()
    return nc


def _get_program(mode, repeat=1):
    key = (mode, repeat)
    if key not in _prog_cache:
        _prog_cache[key] = _build_program(mode, repeat)
    return _prog_cache[key]


def _host_prep(u_t, h_prev, action, w_u, w_h, mode):
    """Host-side layout prep shared by kernel() and the timing harness."""
    np_in = np.float32
    if mode == "bf16":
        import ml_dtypes

        np_in = ml_dtypes.bfloat16

    # host-side circular halo pads (pure layout; all compute stays on device)
    hpad = np.pad(h_prev, ((0, 0), (0, 0), (1, 8), (1, 8)), mode="wrap")
    upad = np.pad(u_t, ((0, 0), (0, 0), (1, 1), (1, 1)), mode="wrap")
    # u im2col layout: uim[b, (di*3+dj)*3+ci, i*64+j] = upad[b, ci, i+di, j+dj]
    uim = np.empty((B, 9 * CIN, WIN * WIN), np.float32)
    for t in range(9):
        di, dj = t // 3, t % 3
        uim[:, 3 * t : 3 * t + 3, :] = upad[
            :, :, di : di + WIN, dj : dj + WIN
        ].reshape(B, CIN, WIN * WIN)
    # weight layouts for matmul lhsT (stationary [K, M])
    wh_l = np.ascontiguousarray(
        w_h.transpose(1, 2, 3, 0).reshape(CH, 9 * CH)
    )  # [ci, (di*3+dj)*128+co]
    wu_l = np.ascontiguousarray(
        w_u.transpose(2, 3, 1, 0).reshape(9 * CIN, CH)
    )  # [(di*3+dj)*3+ci, co]

    if mode == "f32r":
        hpad = _round_tf32(hpad)
        uim = _round_tf32(uim)
        wh_l = _round_tf32(wh_l)
        wu_l = _round_tf32(wu_l)
    else:
        hpad = hpad.astype(np_in)
        uim = uim.astype(np_in)
        wh_l = wh_l.astype(np_in)
        wu_l = wu_l.astype(np_in)
    return {"hpad": hpad, "uim": uim, "act": action, "wh": wh_l, "wu": wu_l}


def _core_in_map(prep, c):
    s = slice(c * BLOC, (c + 1) * BLOC)
    return {
        "h": np.ascontiguousarray(prep["hpad"][s]).reshape(BLOC * CH, HP, HP),
        "u": np.ascontiguousarray(prep["uim"][s]).reshape(
            BLOC * 9 * CIN, WIN * WIN
        ),
        "act": np.ascontiguousarray(prep["act"][s]).reshape(1, 2 * BLOC),
        "wh": prep["wh"],
        "wu": prep["wu"],
    }


def kernel(u_t, h_prev, action, w_u, w_h):
    global LAST_RESULTS
    from concourse.bass_utils import run_bass_kernel_spmd

    mode = MODE
    nc = _get_program(mode)

    u_t = np.asarray(u_t, np.float32)
    h_prev = np.asarray(h_prev, np.float32)
    action = np.asarray(action, np.int32)
    w_u = np.asarray(w_u, np.float32)
    w_h = np.asarray(w_h, np.float32)

    prep = _host_prep(u_t, h_prev, action, w_u, w_h, mode)
    in_maps = [_core_in_map(prep, c) for c in range(NCORES)]

    res = run_bass_kernel_spmd(nc, in_maps, list(range(NCORES)), trace=TRACE)
    LAST_RESULTS = res
    out = np.concatenate(
        [
            np.asarray(r["out"], np.float32).reshape(BLOC, CH, WORLD, WORLD)
            for r in res.results
        ],
        axis=0,
    )
    return out

